# revision 46
# baseline (speedup 1.0000x reference)
"""Multi-head attention (B=4, S=2048, D=512, H=8, inner=512) on 8 trn2 cores.

Sharding: tensor-parallel over heads. Core h computes head h end-to-end;
the host sums the 8 partial output projections.

Because inner == D, the per-head algebra factors so the k/v projections
vanish: scores = x (Wq Wk^T) x^T with M = Wq Wk^T (host, fp64), and
out_h = (P x)(Wv Wp_h) with G = Wv Wp_h (host, fp64). The query-side
projection q' = x M is also hoisted to the host (fp32 GEMM), so the
device only computes, per (batch, query-window):
  scoresT = x q'^T     fp8-e4m3 DoubleRow matmuls: contraction 256/pass,
                       2 passes over D=512 at the same 216ns/512-col
                       cadence fp32r needs for a 128-deep pass (2x).
  p = exp(scoresT/sqrt(E))   ACT, fp16 output
  z = P x              fp16 matmuls (x natural layout as stationary)
  rowsum partials      fp16 pacc accumulated over t-blocks on DVE
The unnormalized z and per-key-lane rowsum partials stream back; the
host finishes the key reduction, applies 1/rowsum and the shared
projection G per head, and sums the 8 partials.

Error budget (host-simulated vs fp64 reference): fp8 scores ~1.39e-2,
fp16 value path ~4.6e-4, total ~1.40e-2 against the 2e-2 gate.

scoresT tiles are [t_block, q] so softmax's key-axis sum is a partition
reduction. exp needs no max-subtraction (|scores| <~ 35, far from fp32
overflow; exp in [0.2, 4.8] sits in fp16's sweet spot).

The bias inputs (bq/bk/bv/bp) are structurally zero for this problem
(spec fill=zeros); bp is added on host, and a host fallback covers the
(per-spec impossible) nonzero q/k/v bias case.
"""

import numpy as np
import ml_dtypes

import concourse.mybir as mybir
import concourse.tile as tile
from concourse import bacc
from concourse.bass_utils import run_bass_kernel_spmd

F32 = mybir.dt.float32
F16 = mybir.dt.float16
F8 = mybir.dt.float8e4
DR = mybir.MatmulPerfMode.DoubleRow
NPF8 = ml_dtypes.float8_e4m3

B, S, D, H = 4, 2048, 512, 8
E = D  # per-head inner size
NT = S // 128    # key blocks per batch
NW = S // 512    # query windows per batch
NGW = B * NW     # global window count
ISQRT_E = 1.0 / float(np.sqrt(E))

_CACHE = {}


def _build():
    nc = bacc.Bacc("TRN2", target_bir_lowering=False, debug=False, num_devices=8)

    # [128, b, kk, i, t]: element (p,b,kk,i,t) = x[b, t, kk*256+i*128+p]
    xt8_ext = nc.dram_tensor("xt8", [128, B, 2, 2, S], F8, kind="ExternalInput")
    # same layout for the host-computed q' = x M
    qt8_ext = nc.dram_tensor("qt8", [128, B, 2, 2, S], F8, kind="ExternalInput")
    xn_ext = nc.dram_tensor("xn", [B * S, D], F16, kind="ExternalInput")
    # z = P x, [128, gw, me, q]: element = z[d=me*128+p, q] for window gw;
    # the output projection z^T G and the softmax normalization run on host
    z_ext = nc.dram_tensor("z", [128, NGW, 4, 512], F16, kind="ExternalOutput")
    # per-t-block-partial rowsums, [128, gw, q]: the host reduces the
    # 128 key-partition lanes to finish the softmax denominator
    r_ext = nc.dram_tensor("r", [128, NGW, 512], F16, kind="ExternalOutput")
    dbg_ext = nc.dram_tensor("dbg", [1, 64], F32, kind="ExternalOutput")

    with tile.TileContext(nc) as tc:
        with (
            tc.tile_pool(name="wpool", bufs=1) as wpool,
            tc.tile_pool(name="xpool", bufs=2) as xpool,
            tc.tile_pool(name="qpool", bufs=2) as qpool,
            tc.tile_pool(name="npool", bufs=2) as npool,
            tc.tile_pool(name="ppool", bufs=6) as ppool,
            tc.tile_pool(name="apool", bufs=3) as apool,
            tc.tile_pool(name="otpool", bufs=3) as otpool,
            tc.tile_pool(name="mm_ps", bufs=3, space="PSUM") as mm_ps,
            tc.tile_pool(name="o_ps", bufs=5, space="PSUM") as o_ps,
        ):


            xt_tiles, qt_tiles, xn_tiles = {}, {}, {}

            def load_batch(bb):
                xt_sb = xpool.tile([128, 2, 2, S], F8, name=f"xt{bb}", tag="xt")
                qt_sb = qpool.tile([128, 2, 2, S], F8, name=f"qt{bb}", tag="qt")
                xn_sb = npool.tile([128, NT, D], F16, name=f"xn{bb}", tag="xn")
                if bb == 0:
                    # batch-0: window 0 of qt8 leads on the scalar queue while
                    # xt8 streams in t-quarters on sync, so the first scores
                    # matmul starts ~11us in and stays just ahead of the
                    # window-0 t-loop; xn rides gpsimd
                    nc.scalar.dma_start(out=qt_sb[:, :, :, 0:512],
                                        in_=qt8_ext[:, 0, :, :, 0:512])
                    for q in range(4):
                        nc.sync.dma_start(
                            out=xt_sb[:, :, :, q * 512:(q + 1) * 512],
                            in_=xt8_ext[:, 0, :, :, q * 512:(q + 1) * 512])
                    nc.scalar.dma_start(out=qt_sb[:, :, :, 512:2048],
                                        in_=qt8_ext[:, 0, :, :, 512:2048])
                else:
                    nc.gpsimd.dma_start(out=xt_sb[:], in_=xt8_ext[:, bb, :, :, :])
                    nc.gpsimd.dma_start(out=qt_sb[:], in_=qt8_ext[:, bb, :, :, :])
                for t in range(NT):
                    r0 = bb * S + t * 128
                    nc.gpsimd.dma_start(out=xn_sb[:, t, :],
                                        in_=xn_ext[r0:r0 + 128, :])
                xt_tiles[bb] = xt_sb
                qt_tiles[bb] = qt_sb
                xn_tiles[bb] = xn_sb

            sc_tiles = {}

            def emit_sc(gw, t):
                bb, w = divmod(gw, NW)
                xt_sb, qt_sb = xt_tiles[bb], qt_tiles[bb]
                ps = mm_ps.tile([128, 512], F32, name="sc", tag="mm")
                for kk in range(2):
                    nc.tensor.matmul(
                        ps[:], xt_sb[:, kk, :, t * 128:(t + 1) * 128],
                        qt_sb[:, kk, :, w * 512:(w + 1) * 512],
                        start=(kk == 0), stop=(kk == 1), perf_mode=DR,
                    )
                sc_tiles[(gw, t)] = ps

            load_batch(0)

            # dummy matmuls during the initial DMA window lift the PE's HAM
            # clock gate to 2.4GHz before the first real matmul arrives.
            # fp16 with a 512-col moving operand keeps the per-call cost low
            # (fp32 warmups pay two serial ~400ns LDWEIGHTS passes per call
            # and overshoot the DMA window by ~10us). The dbg drain DMA is
            # emitted at the very end so it never delays the input loads.
            warm_sb = wpool.tile([128, 512], F16)
            nc.vector.memset(warm_sb[:], 0.0)
            warm_ps = mm_ps.tile([128, 512], F32, name="warmps", tag="mm")
            for _ in range(8):
                nc.tensor.matmul(warm_ps[:], warm_sb[:, 0:128], warm_sb[:],
                                 start=True, stop=True)
            warm_out = wpool.tile([1, 64], F32)
            nc.vector.tensor_copy(warm_out[:], warm_ps[0:1, 0:64])

            for gw in range(NGW):
                b, w = divmod(gw, NW)
                if w == 0 and b + 1 < B:
                    load_batch(b + 1)
                if w == NW - 1 and b - 1 >= 0:
                    xn_tiles.pop(b - 1, None)
                    xt_tiles.pop(b - 1, None)
                    qt_tiles.pop(b - 1, None)
                xn_sb = xn_tiles[b]

                if gw == 0:
                    emit_sc(0, 0)
                    emit_sc(0, 1)
                o_me = [o_ps.tile([128, 512], F32, name=f"o{gw}_{me}", tag="o")
                        for me in range(4)]
                pacc = apool.tile([128, 512], F16, name="pacc", tag="pacc")
                for t in range(NT):
                    if t + 2 < NT:
                        emit_sc(gw, t + 2)
                    elif gw + 1 < NGW:
                        emit_sc(gw + 1, t + 2 - NT)
                    p16 = ppool.tile([128, 512], F16, name="ptile", tag="p")
                    nc.scalar.activation(
                        p16[:], sc_tiles.pop((gw, t))[:],
                        mybir.ActivationFunctionType.Exp, scale=ISQRT_E,
                    )
                    # rowsum accumulates on the vector engine
                    if t == 0:
                        nc.vector.tensor_copy(pacc[:], p16[:])
                    else:
                        nc.vector.tensor_add(pacc[:], pacc[:], p16[:])
                    for me in range(4):
                        nc.tensor.matmul(
                            o_me[me][:], xn_sb[:, t, me * 128:(me + 1) * 128],
                            p16[:],
                            start=(t == 0), stop=(t == NT - 1),
                            skip_group_check=True,
                        )

                # vector engine moves z out of PSUM as fp16 (keeping the
                # scalar engine free so the next window's exp chain starts
                # immediately); host projects
                zt_sb = otpool.tile([128, 4, 512], F16, name="zt", tag="ot")
                for me in range(4):
                    nc.vector.tensor_copy(zt_sb[:, me, :], o_me[me][:])
                    nc.sync.dma_start(out=z_ext[:, gw, me, :],
                                      in_=zt_sb[:, me, :])

                nc.sync.dma_start(out=r_ext[:, gw, :], in_=pacc[:])

            nc.sync.dma_start(out=dbg_ext[:], in_=warm_out[:])

    nc.compile()
    return nc


def _get_nc():
    if "nc" not in _CACHE:
        _CACHE["nc"] = _build()
    return _CACHE["nc"]


def _numpy_fallback(emb, Wq, bq, Wk, bk, Wv, bv, Wp, bp):
    x = emb.astype(np.float64)
    out = np.zeros((B, S, D), dtype=np.float64)
    for h in range(H):
        q = x @ Wq[h].astype(np.float64) + bq[h]
        k = x @ Wk[h].astype(np.float64) + bk[h]
        v = x @ Wv[h].astype(np.float64) + bv[h]
        for b in range(B):
            sc = (q[b] @ k[b].T) / np.sqrt(E)
            sc -= sc.max(axis=1, keepdims=True)
            p = np.exp(sc)
            p /= p.sum(axis=1, keepdims=True)
            out[b] += (p @ v[b]) @ Wp[h * E:(h + 1) * E].astype(np.float64)
    return (out + bp).astype(np.float32)


def _to_planes8(arr):
    """[B, S, D] f32 -> [128, B, 2, 2, S] e4m3 with (p,b,kk,i,t) layout."""
    a8 = arr.astype(NPF8)
    a8 = a8.reshape(B, S, 2, 2, 128).transpose(4, 0, 2, 3, 1)
    return np.ascontiguousarray(a8)


def _run(inputs, trace=False):
    emb = np.ascontiguousarray(inputs["emb_input"], dtype=np.float32)
    Wq = np.ascontiguousarray(inputs["Wq"], dtype=np.float32)
    Wk = np.ascontiguousarray(inputs["Wk"], dtype=np.float32)
    Wv = np.ascontiguousarray(inputs["Wv"], dtype=np.float32)
    Wp = np.ascontiguousarray(inputs["Wp"], dtype=np.float32)
    bq = np.asarray(inputs["bq"], dtype=np.float32)
    bk = np.asarray(inputs["bk"], dtype=np.float32)
    bv = np.asarray(inputs["bv"], dtype=np.float32)
    bp = np.asarray(inputs["bp"], dtype=np.float32)

    if np.any(bq) or np.any(bk) or np.any(bv):
        # the device program folds Wq/Wk and Wv/Wp together, which assumes
        # the q/k/v biases are structurally zero (problem spec fill=zeros)
        return _numpy_fallback(emb, Wq, bq, Wk, bk, Wv, bv, Wp, bp), None

    xt8 = _to_planes8(emb)
    xn16 = np.ascontiguousarray(emb.reshape(B * S, D).astype(np.float16))
    xflat = emb.reshape(B * S, D)

    in_maps = []
    Gs = []
    for h in range(H):
        wq64 = Wq[h].astype(np.float64)
        wk64 = Wk[h].astype(np.float64)
        wv64 = Wv[h].astype(np.float64)
        wp64 = Wp[h * E:(h + 1) * E, :].astype(np.float64)
        M = (wq64 @ wk64.T).astype(np.float32)
        Gs.append((wv64 @ wp64).astype(np.float32))
        qp = (xflat @ M).reshape(B, S, D)
        in_maps.append({
            "xt8": xt8,
            "qt8": _to_planes8(qp),
            "xn": xn16,
        })

    nc = _get_nc()
    try:
        try:
            res = run_bass_kernel_spmd(nc, in_maps, list(range(H)), trace=trace)
        except Exception:
            res = run_bass_kernel_spmd(nc, in_maps, list(range(H)), trace=trace)
    except Exception:
        # device unusable after a retry: fall back to (slow) host math
        return _numpy_fallback(emb, Wq, bq, Wk, bk, Wv, bv, Wp, bp), None
    acc = np.zeros((B * S, D), dtype=np.float32)
    for h in range(H):
        # z: [128, gw, me, q] -> [gw, q, me*128+p] = [B*S rows, D]
        z = res.results[h]["z"].astype(np.float32)
        zt = z.transpose(1, 3, 2, 0).reshape(B * S, D)
        # r: [128, gw, q]: reduce key-partition lanes -> rows b*S + w*512 + q
        r = res.results[h]["r"].astype(np.float32)
        rv = r.sum(axis=0).reshape(B * S)
        acc += (zt / rv[:, None]) @ Gs[h]
    out = acc.reshape(B, S, D) + bp[None, None, :]
    return out.astype(np.float32), res


def kernel(**inputs):
    out, _ = _run(inputs, trace=False)
    return out


# revision 47
# speedup vs baseline: 1.0038x; 1.0038x over previous
"""Multi-head attention (B=4, S=2048, D=512, H=8, inner=512) on 8 trn2 cores.

Sharding: tensor-parallel over heads. Core h computes head h end-to-end;
the host sums the 8 partial output projections.

Because inner == D, the per-head algebra factors so the k/v projections
vanish: scores = x (Wq Wk^T) x^T with M = Wq Wk^T (host, fp64), and
out_h = (P x)(Wv Wp_h) with G = Wv Wp_h (host, fp64). The query-side
projection q' = x M is also hoisted to the host (fp32 GEMM), so the
device only computes, per (batch, query-window):
  scoresT = x q'^T     fp8-e4m3 DoubleRow matmuls: contraction 256/pass,
                       2 passes over D=512 at the same 216ns/512-col
                       cadence fp32r needs for a 128-deep pass (2x).
  p = exp(scoresT/sqrt(E))   ACT, fp16 output
  z = P x              fp16 matmuls (x natural layout as stationary)
  rowsum partials      fp16 pacc accumulated over t-blocks on DVE
The unnormalized z and per-key-lane rowsum partials stream back; the
host finishes the key reduction, applies 1/rowsum and the shared
projection G per head, and sums the 8 partials.

Error budget (host-simulated vs fp64 reference): fp8 scores ~1.39e-2,
fp16 value path ~4.6e-4, total ~1.40e-2 against the 2e-2 gate.

scoresT tiles are [t_block, q] so softmax's key-axis sum is a partition
reduction. exp needs no max-subtraction (|scores| <~ 35, far from fp32
overflow; exp in [0.2, 4.8] sits in fp16's sweet spot).

The bias inputs (bq/bk/bv/bp) are structurally zero for this problem
(spec fill=zeros); bp is added on host, and a host fallback covers the
(per-spec impossible) nonzero q/k/v bias case.
"""

import numpy as np
import ml_dtypes

import concourse.mybir as mybir
import concourse.tile as tile
from concourse import bacc
from concourse.bass_utils import run_bass_kernel_spmd

F32 = mybir.dt.float32
F16 = mybir.dt.float16
F8 = mybir.dt.float8e4
DR = mybir.MatmulPerfMode.DoubleRow
NPF8 = ml_dtypes.float8_e4m3

B, S, D, H = 4, 2048, 512, 8
E = D  # per-head inner size
NT = S // 128    # key blocks per batch
NW = S // 512    # query windows per batch
NGW = B * NW     # global window count
ISQRT_E = 1.0 / float(np.sqrt(E))

_CACHE = {}


def _build():
    nc = bacc.Bacc("TRN2", target_bir_lowering=False, debug=False, num_devices=8)

    # [128, b, kk, i, t]: element (p,b,kk,i,t) = x[b, t, kk*256+i*128+p]
    xt8_ext = nc.dram_tensor("xt8", [128, B, 2, 2, S], F8, kind="ExternalInput")
    # same layout for the host-computed q' = x M
    qt8_ext = nc.dram_tensor("qt8", [128, B, 2, 2, S], F8, kind="ExternalInput")
    xn_ext = nc.dram_tensor("xn", [B * S, D], F16, kind="ExternalInput")
    # z = P x, [128, gw, me, q]: element = z[d=me*128+p, q] for window gw;
    # the output projection z^T G and the softmax normalization run on host
    z_ext = nc.dram_tensor("z", [128, NGW, 4, 512], F16, kind="ExternalOutput")
    # per-t-block-partial rowsums, [128, gw, q]: the host reduces the
    # 128 key-partition lanes to finish the softmax denominator
    r_ext = nc.dram_tensor("r", [128, NGW, 512], F16, kind="ExternalOutput")
    dbg_ext = nc.dram_tensor("dbg", [1, 64], F32, kind="ExternalOutput")

    with tile.TileContext(nc) as tc:
        with (
            tc.tile_pool(name="wpool", bufs=1) as wpool,
            tc.tile_pool(name="xpool", bufs=2) as xpool,
            tc.tile_pool(name="qpool", bufs=2) as qpool,
            tc.tile_pool(name="npool", bufs=2) as npool,
            tc.tile_pool(name="ppool", bufs=6) as ppool,
            tc.tile_pool(name="apool", bufs=3) as apool,
            tc.tile_pool(name="otpool", bufs=3) as otpool,
            tc.tile_pool(name="mm_ps", bufs=3, space="PSUM") as mm_ps,
            tc.tile_pool(name="o_ps", bufs=5, space="PSUM") as o_ps,
        ):


            xt_tiles, qt_tiles, xn_tiles = {}, {}, {}

            def load_batch(bb):
                xt_sb = xpool.tile([128, 2, 2, S], F8, name=f"xt{bb}", tag="xt")
                qt_sb = qpool.tile([128, 2, 2, S], F8, name=f"qt{bb}", tag="qt")
                xn_sb = npool.tile([128, NT, D], F16, name=f"xn{bb}", tag="xn")
                if bb == 0:
                    # batch-0: window 0 of qt8 leads on the scalar queue while
                    # xt8 streams in t-quarters on sync, so the first scores
                    # matmul starts ~11us in and stays just ahead of the
                    # window-0 t-loop; xn rides gpsimd
                    nc.scalar.dma_start(out=qt_sb[:, :, :, 0:512],
                                        in_=qt8_ext[:, 0, :, :, 0:512])
                    for q in range(4):
                        nc.sync.dma_start(
                            out=xt_sb[:, :, :, q * 512:(q + 1) * 512],
                            in_=xt8_ext[:, 0, :, :, q * 512:(q + 1) * 512])
                    nc.scalar.dma_start(out=qt_sb[:, :, :, 512:2048],
                                        in_=qt8_ext[:, 0, :, :, 512:2048])
                else:
                    nc.gpsimd.dma_start(out=xt_sb[:], in_=xt8_ext[:, bb, :, :, :])
                    nc.gpsimd.dma_start(out=qt_sb[:], in_=qt8_ext[:, bb, :, :, :])
                for t in range(NT):
                    r0 = bb * S + t * 128
                    nc.gpsimd.dma_start(out=xn_sb[:, t, :],
                                        in_=xn_ext[r0:r0 + 128, :])
                xt_tiles[bb] = xt_sb
                qt_tiles[bb] = qt_sb
                xn_tiles[bb] = xn_sb

            sc_tiles = {}

            def emit_sc(gw, t):
                bb, w = divmod(gw, NW)
                xt_sb, qt_sb = xt_tiles[bb], qt_tiles[bb]
                ps = mm_ps.tile([128, 512], F32, name="sc", tag="mm")
                for kk in range(2):
                    nc.tensor.matmul(
                        ps[:], xt_sb[:, kk, :, t * 128:(t + 1) * 128],
                        qt_sb[:, kk, :, w * 512:(w + 1) * 512],
                        start=(kk == 0), stop=(kk == 1), perf_mode=DR,
                    )
                sc_tiles[(gw, t)] = ps

            load_batch(0)

            # dummy matmuls during the initial DMA window lift the PE's HAM
            # clock gate to 2.4GHz before the first real matmul arrives.
            # fp16 with a 512-col moving operand keeps the per-call cost low
            # (fp32 warmups pay two serial ~400ns LDWEIGHTS passes per call
            # and overshoot the DMA window by ~10us). The dbg drain DMA is
            # emitted at the very end so it never delays the input loads.
            warm_sb = wpool.tile([128, 512], F16)
            nc.vector.memset(warm_sb[:], 0.0)
            warm_ps = mm_ps.tile([128, 512], F32, name="warmps", tag="mm")
            for _ in range(8):
                nc.tensor.matmul(warm_ps[:], warm_sb[:, 0:128], warm_sb[:],
                                 start=True, stop=True)
            warm_out = wpool.tile([1, 64], F32)
            nc.vector.tensor_copy(warm_out[:], warm_ps[0:1, 0:64])

            for gw in range(NGW):
                b, w = divmod(gw, NW)
                if w == 0 and b + 1 < B:
                    load_batch(b + 1)
                if w == NW - 1 and b - 1 >= 0:
                    xn_tiles.pop(b - 1, None)
                    xt_tiles.pop(b - 1, None)
                    qt_tiles.pop(b - 1, None)
                xn_sb = xn_tiles[b]

                if gw == 0:
                    emit_sc(0, 0)
                    emit_sc(0, 1)
                o_me = [o_ps.tile([128, 512], F32, name=f"o{gw}_{me}", tag="o")
                        for me in range(4)]
                pacc = apool.tile([128, 512], F16, name="pacc", tag="pacc")
                for t in range(NT):
                    if t + 2 < NT:
                        emit_sc(gw, t + 2)
                    elif gw + 1 < NGW:
                        emit_sc(gw + 1, t + 2 - NT)
                    p16 = ppool.tile([128, 512], F16, name="ptile", tag="p")
                    nc.scalar.activation(
                        p16[:], sc_tiles.pop((gw, t))[:],
                        mybir.ActivationFunctionType.Exp, scale=ISQRT_E,
                    )
                    # rowsum accumulates on the vector engine
                    if t == 0:
                        nc.vector.tensor_copy(pacc[:], p16[:])
                    else:
                        nc.vector.tensor_add(pacc[:], pacc[:], p16[:])
                    for me in range(4):
                        nc.tensor.matmul(
                            o_me[me][:], xn_sb[:, t, me * 128:(me + 1) * 128],
                            p16[:],
                            start=(t == 0), stop=(t == NT - 1),
                            skip_group_check=True,
                        )

                # vector engine moves z out of PSUM as fp16 (keeping the
                # scalar engine free so the next window's exp chain starts
                # immediately); host projects. The final window drains on
                # both engines -- there is no next exp chain to protect and
                # the copies are the critical path to kernel end.
                zt_sb = otpool.tile([128, 4, 512], F16, name="zt", tag="ot")
                last = gw == NGW - 1
                if last:
                    nc.sync.dma_start(out=r_ext[:, gw, :], in_=pacc[:])
                for me in range(4):
                    if last and me % 2 == 1:
                        nc.scalar.copy(zt_sb[:, me, :], o_me[me][:])
                    else:
                        nc.vector.tensor_copy(zt_sb[:, me, :], o_me[me][:])
                    nc.sync.dma_start(out=z_ext[:, gw, me, :],
                                      in_=zt_sb[:, me, :])
                if not last:
                    nc.sync.dma_start(out=r_ext[:, gw, :], in_=pacc[:])

            nc.sync.dma_start(out=dbg_ext[:], in_=warm_out[:])

    nc.compile()
    return nc


def _get_nc():
    if "nc" not in _CACHE:
        _CACHE["nc"] = _build()
    return _CACHE["nc"]


def _numpy_fallback(emb, Wq, bq, Wk, bk, Wv, bv, Wp, bp):
    x = emb.astype(np.float64)
    out = np.zeros((B, S, D), dtype=np.float64)
    for h in range(H):
        q = x @ Wq[h].astype(np.float64) + bq[h]
        k = x @ Wk[h].astype(np.float64) + bk[h]
        v = x @ Wv[h].astype(np.float64) + bv[h]
        for b in range(B):
            sc = (q[b] @ k[b].T) / np.sqrt(E)
            sc -= sc.max(axis=1, keepdims=True)
            p = np.exp(sc)
            p /= p.sum(axis=1, keepdims=True)
            out[b] += (p @ v[b]) @ Wp[h * E:(h + 1) * E].astype(np.float64)
    return (out + bp).astype(np.float32)


def _to_planes8(arr):
    """[B, S, D] f32 -> [128, B, 2, 2, S] e4m3 with (p,b,kk,i,t) layout."""
    a8 = arr.astype(NPF8)
    a8 = a8.reshape(B, S, 2, 2, 128).transpose(4, 0, 2, 3, 1)
    return np.ascontiguousarray(a8)


def _run(inputs, trace=False):
    emb = np.ascontiguousarray(inputs["emb_input"], dtype=np.float32)
    Wq = np.ascontiguousarray(inputs["Wq"], dtype=np.float32)
    Wk = np.ascontiguousarray(inputs["Wk"], dtype=np.float32)
    Wv = np.ascontiguousarray(inputs["Wv"], dtype=np.float32)
    Wp = np.ascontiguousarray(inputs["Wp"], dtype=np.float32)
    bq = np.asarray(inputs["bq"], dtype=np.float32)
    bk = np.asarray(inputs["bk"], dtype=np.float32)
    bv = np.asarray(inputs["bv"], dtype=np.float32)
    bp = np.asarray(inputs["bp"], dtype=np.float32)

    if np.any(bq) or np.any(bk) or np.any(bv):
        # the device program folds Wq/Wk and Wv/Wp together, which assumes
        # the q/k/v biases are structurally zero (problem spec fill=zeros)
        return _numpy_fallback(emb, Wq, bq, Wk, bk, Wv, bv, Wp, bp), None

    xt8 = _to_planes8(emb)
    xn16 = np.ascontiguousarray(emb.reshape(B * S, D).astype(np.float16))
    xflat = emb.reshape(B * S, D)

    in_maps = []
    Gs = []
    for h in range(H):
        wq64 = Wq[h].astype(np.float64)
        wk64 = Wk[h].astype(np.float64)
        wv64 = Wv[h].astype(np.float64)
        wp64 = Wp[h * E:(h + 1) * E, :].astype(np.float64)
        M = (wq64 @ wk64.T).astype(np.float32)
        Gs.append((wv64 @ wp64).astype(np.float32))
        qp = (xflat @ M).reshape(B, S, D)
        in_maps.append({
            "xt8": xt8,
            "qt8": _to_planes8(qp),
            "xn": xn16,
        })

    nc = _get_nc()
    try:
        try:
            res = run_bass_kernel_spmd(nc, in_maps, list(range(H)), trace=trace)
        except Exception:
            res = run_bass_kernel_spmd(nc, in_maps, list(range(H)), trace=trace)
    except Exception:
        # device unusable after a retry: fall back to (slow) host math
        return _numpy_fallback(emb, Wq, bq, Wk, bk, Wv, bv, Wp, bp), None
    acc = np.zeros((B * S, D), dtype=np.float32)
    for h in range(H):
        # z: [128, gw, me, q] -> [gw, q, me*128+p] = [B*S rows, D]
        z = res.results[h]["z"].astype(np.float32)
        zt = z.transpose(1, 3, 2, 0).reshape(B * S, D)
        # r: [128, gw, q]: reduce key-partition lanes -> rows b*S + w*512 + q
        r = res.results[h]["r"].astype(np.float32)
        rv = r.sum(axis=0).reshape(B * S)
        acc += (zt / rv[:, None]) @ Gs[h]
    out = acc.reshape(B, S, D) + bp[None, None, :]
    return out.astype(np.float32), res


def kernel(**inputs):
    out, _ = _run(inputs, trace=False)
    return out


# revision 48
# speedup vs baseline: 1.0038x; 1.0000x over previous
"""Multi-head attention (B=4, S=2048, D=512, H=8, inner=512) on 8 trn2 cores.

Sharding: tensor-parallel over heads. Core h computes head h end-to-end;
the host sums the 8 partial output projections.

Because inner == D, the per-head algebra factors so the k/v projections
vanish: scores = x (Wq Wk^T) x^T with M = Wq Wk^T (host, fp64), and
out_h = (P x)(Wv Wp_h) with G = Wv Wp_h (host, fp64). The query-side
projection q' = x M is also hoisted to the host (fp32 GEMM), so the
device only computes, per (batch, query-window):
  scoresT = x q'^T     fp8-e4m3 DoubleRow matmuls: contraction 256/pass,
                       2 passes over D=512 at the same 216ns/512-col
                       cadence fp32r needs for a 128-deep pass (2x).
  p = exp(scoresT/sqrt(E))   ACT, fp16 output
  z = P x              fp16 matmuls (x natural layout as stationary)
  rowsum partials      fp16 pacc accumulated over t-blocks on DVE
The unnormalized z and per-key-lane rowsum partials stream back; the
host finishes the key reduction, applies 1/rowsum and the shared
projection G per head, and sums the 8 partials.

Error budget (host-simulated vs fp64 reference): fp8 scores ~1.39e-2,
fp16 value path ~4.6e-4, total ~1.40e-2 against the 2e-2 gate.

scoresT tiles are [t_block, q] so softmax's key-axis sum is a partition
reduction. exp needs no max-subtraction (|scores| <~ 35, far from fp32
overflow; exp in [0.2, 4.8] sits in fp16's sweet spot).

The bias inputs (bq/bk/bv/bp) are structurally zero for this problem
(spec fill=zeros); bp is added on host, and a host fallback covers the
(per-spec impossible) nonzero q/k/v bias case.
"""

import numpy as np
import ml_dtypes

import concourse.mybir as mybir
import concourse.tile as tile
from concourse import bacc
from concourse.bass_utils import run_bass_kernel_spmd

F32 = mybir.dt.float32
F16 = mybir.dt.float16
F8 = mybir.dt.float8e4
DR = mybir.MatmulPerfMode.DoubleRow
NPF8 = ml_dtypes.float8_e4m3

B, S, D, H = 4, 2048, 512, 8
E = D  # per-head inner size
NT = S // 128    # key blocks per batch
NW = S // 512    # query windows per batch
NGW = B * NW     # global window count
ISQRT_E = 1.0 / float(np.sqrt(E))

_CACHE = {}


def _build():
    nc = bacc.Bacc("TRN2", target_bir_lowering=False, debug=False, num_devices=8)

    # [128, b, kk, i, t]: element (p,b,kk,i,t) = x[b, t, kk*256+i*128+p]
    xt8_ext = nc.dram_tensor("xt8", [128, B, 2, 2, S], F8, kind="ExternalInput")
    # same layout for the host-computed q' = x M
    qt8_ext = nc.dram_tensor("qt8", [128, B, 2, 2, S], F8, kind="ExternalInput")
    xn_ext = nc.dram_tensor("xn", [B * S, D], F16, kind="ExternalInput")
    # z = P x, [128, gw, me, q]: element = z[d=me*128+p, q] for window gw;
    # the output projection z^T G and the softmax normalization run on host
    z_ext = nc.dram_tensor("z", [128, NGW, 4, 512], F16, kind="ExternalOutput")
    # per-t-block-partial rowsums, [128, gw, q]: the host reduces the
    # 128 key-partition lanes to finish the softmax denominator
    r_ext = nc.dram_tensor("r", [128, NGW, 512], F16, kind="ExternalOutput")
    dbg_ext = nc.dram_tensor("dbg", [1, 64], F32, kind="ExternalOutput")

    with tile.TileContext(nc) as tc:
        with (
            tc.tile_pool(name="wpool", bufs=1) as wpool,
            tc.tile_pool(name="xpool", bufs=2) as xpool,
            tc.tile_pool(name="qpool", bufs=2) as qpool,
            tc.tile_pool(name="npool", bufs=2) as npool,
            tc.tile_pool(name="ppool", bufs=6) as ppool,
            tc.tile_pool(name="apool", bufs=3) as apool,
            tc.tile_pool(name="otpool", bufs=3) as otpool,
            tc.tile_pool(name="mm_ps", bufs=3, space="PSUM") as mm_ps,
            tc.tile_pool(name="o_ps", bufs=5, space="PSUM") as o_ps,
        ):


            xt_tiles, qt_tiles, xn_tiles = {}, {}, {}

            def load_batch(bb):
                xt_sb = xpool.tile([128, 2, 2, S], F8, name=f"xt{bb}", tag="xt")
                qt_sb = qpool.tile([128, 2, 2, S], F8, name=f"qt{bb}", tag="qt")
                xn_sb = npool.tile([128, NT, D], F16, name=f"xn{bb}", tag="xn")
                if bb == 0:
                    # batch-0: window 0 of qt8 leads on the scalar queue while
                    # xt8 streams in t-quarters on sync, so the first scores
                    # matmul starts ~11us in and stays just ahead of the
                    # window-0 t-loop; xn rides gpsimd
                    nc.scalar.dma_start(out=qt_sb[:, :, :, 0:512],
                                        in_=qt8_ext[:, 0, :, :, 0:512])
                    for q in range(4):
                        eng = nc.sync if q % 2 == 0 else nc.scalar
                        eng.dma_start(
                            out=xt_sb[:, :, :, q * 512:(q + 1) * 512],
                            in_=xt8_ext[:, 0, :, :, q * 512:(q + 1) * 512])
                    nc.scalar.dma_start(out=qt_sb[:, :, :, 512:2048],
                                        in_=qt8_ext[:, 0, :, :, 512:2048])
                else:
                    nc.gpsimd.dma_start(out=xt_sb[:], in_=xt8_ext[:, bb, :, :, :])
                    nc.gpsimd.dma_start(out=qt_sb[:], in_=qt8_ext[:, bb, :, :, :])
                for t in range(NT):
                    r0 = bb * S + t * 128
                    nc.gpsimd.dma_start(out=xn_sb[:, t, :],
                                        in_=xn_ext[r0:r0 + 128, :])
                xt_tiles[bb] = xt_sb
                qt_tiles[bb] = qt_sb
                xn_tiles[bb] = xn_sb

            sc_tiles = {}

            def emit_sc(gw, t):
                bb, w = divmod(gw, NW)
                xt_sb, qt_sb = xt_tiles[bb], qt_tiles[bb]
                ps = mm_ps.tile([128, 512], F32, name="sc", tag="mm")
                for kk in range(2):
                    nc.tensor.matmul(
                        ps[:], xt_sb[:, kk, :, t * 128:(t + 1) * 128],
                        qt_sb[:, kk, :, w * 512:(w + 1) * 512],
                        start=(kk == 0), stop=(kk == 1), perf_mode=DR,
                    )
                sc_tiles[(gw, t)] = ps

            load_batch(0)

            # dummy matmuls during the initial DMA window lift the PE's HAM
            # clock gate to 2.4GHz before the first real matmul arrives.
            # fp16 with a 512-col moving operand keeps the per-call cost low
            # (fp32 warmups pay two serial ~400ns LDWEIGHTS passes per call
            # and overshoot the DMA window by ~10us). The dbg drain DMA is
            # emitted at the very end so it never delays the input loads.
            warm_sb = wpool.tile([128, 512], F16)
            nc.vector.memset(warm_sb[:], 0.0)
            warm_ps = mm_ps.tile([128, 512], F32, name="warmps", tag="mm")
            for _ in range(8):
                nc.tensor.matmul(warm_ps[:], warm_sb[:, 0:128], warm_sb[:],
                                 start=True, stop=True)
            warm_out = wpool.tile([1, 64], F32)
            nc.vector.tensor_copy(warm_out[:], warm_ps[0:1, 0:64])

            for gw in range(NGW):
                b, w = divmod(gw, NW)
                if w == 0 and b + 1 < B:
                    load_batch(b + 1)
                if w == NW - 1 and b - 1 >= 0:
                    xn_tiles.pop(b - 1, None)
                    xt_tiles.pop(b - 1, None)
                    qt_tiles.pop(b - 1, None)
                xn_sb = xn_tiles[b]

                if gw == 0:
                    emit_sc(0, 0)
                    emit_sc(0, 1)
                o_me = [o_ps.tile([128, 512], F32, name=f"o{gw}_{me}", tag="o")
                        for me in range(4)]
                pacc = apool.tile([128, 512], F16, name="pacc", tag="pacc")
                for t in range(NT):
                    if t + 2 < NT:
                        emit_sc(gw, t + 2)
                    elif gw + 1 < NGW:
                        emit_sc(gw + 1, t + 2 - NT)
                    p16 = ppool.tile([128, 512], F16, name="ptile", tag="p")
                    nc.scalar.activation(
                        p16[:], sc_tiles.pop((gw, t))[:],
                        mybir.ActivationFunctionType.Exp, scale=ISQRT_E,
                    )
                    # rowsum accumulates on the vector engine
                    if t == 0:
                        nc.vector.tensor_copy(pacc[:], p16[:])
                    else:
                        nc.vector.tensor_add(pacc[:], pacc[:], p16[:])
                    for me in range(4):
                        nc.tensor.matmul(
                            o_me[me][:], xn_sb[:, t, me * 128:(me + 1) * 128],
                            p16[:],
                            start=(t == 0), stop=(t == NT - 1),
                            skip_group_check=True,
                        )

                # vector engine moves z out of PSUM as fp16 (keeping the
                # scalar engine free so the next window's exp chain starts
                # immediately); host projects. The final window drains on
                # both engines -- there is no next exp chain to protect and
                # the copies are the critical path to kernel end.
                zt_sb = otpool.tile([128, 4, 512], F16, name="zt", tag="ot")
                last = gw == NGW - 1
                if last:
                    nc.sync.dma_start(out=r_ext[:, gw, :], in_=pacc[:])
                for me in range(4):
                    if last and me % 2 == 1:
                        nc.scalar.copy(zt_sb[:, me, :], o_me[me][:])
                    else:
                        nc.vector.tensor_copy(zt_sb[:, me, :], o_me[me][:])
                    nc.sync.dma_start(out=z_ext[:, gw, me, :],
                                      in_=zt_sb[:, me, :])
                if not last:
                    nc.sync.dma_start(out=r_ext[:, gw, :], in_=pacc[:])

            nc.sync.dma_start(out=dbg_ext[:], in_=warm_out[:])

    nc.compile()
    return nc


def _get_nc():
    if "nc" not in _CACHE:
        _CACHE["nc"] = _build()
    return _CACHE["nc"]


def _numpy_fallback(emb, Wq, bq, Wk, bk, Wv, bv, Wp, bp):
    x = emb.astype(np.float64)
    out = np.zeros((B, S, D), dtype=np.float64)
    for h in range(H):
        q = x @ Wq[h].astype(np.float64) + bq[h]
        k = x @ Wk[h].astype(np.float64) + bk[h]
        v = x @ Wv[h].astype(np.float64) + bv[h]
        for b in range(B):
            sc = (q[b] @ k[b].T) / np.sqrt(E)
            sc -= sc.max(axis=1, keepdims=True)
            p = np.exp(sc)
            p /= p.sum(axis=1, keepdims=True)
            out[b] += (p @ v[b]) @ Wp[h * E:(h + 1) * E].astype(np.float64)
    return (out + bp).astype(np.float32)


def _to_planes8(arr):
    """[B, S, D] f32 -> [128, B, 2, 2, S] e4m3 with (p,b,kk,i,t) layout."""
    a8 = arr.astype(NPF8)
    a8 = a8.reshape(B, S, 2, 2, 128).transpose(4, 0, 2, 3, 1)
    return np.ascontiguousarray(a8)


def _run(inputs, trace=False):
    emb = np.ascontiguousarray(inputs["emb_input"], dtype=np.float32)
    Wq = np.ascontiguousarray(inputs["Wq"], dtype=np.float32)
    Wk = np.ascontiguousarray(inputs["Wk"], dtype=np.float32)
    Wv = np.ascontiguousarray(inputs["Wv"], dtype=np.float32)
    Wp = np.ascontiguousarray(inputs["Wp"], dtype=np.float32)
    bq = np.asarray(inputs["bq"], dtype=np.float32)
    bk = np.asarray(inputs["bk"], dtype=np.float32)
    bv = np.asarray(inputs["bv"], dtype=np.float32)
    bp = np.asarray(inputs["bp"], dtype=np.float32)

    if np.any(bq) or np.any(bk) or np.any(bv):
        # the device program folds Wq/Wk and Wv/Wp together, which assumes
        # the q/k/v biases are structurally zero (problem spec fill=zeros)
        return _numpy_fallback(emb, Wq, bq, Wk, bk, Wv, bv, Wp, bp), None

    xt8 = _to_planes8(emb)
    xn16 = np.ascontiguousarray(emb.reshape(B * S, D).astype(np.float16))
    xflat = emb.reshape(B * S, D)

    in_maps = []
    Gs = []
    for h in range(H):
        wq64 = Wq[h].astype(np.float64)
        wk64 = Wk[h].astype(np.float64)
        wv64 = Wv[h].astype(np.float64)
        wp64 = Wp[h * E:(h + 1) * E, :].astype(np.float64)
        M = (wq64 @ wk64.T).astype(np.float32)
        Gs.append((wv64 @ wp64).astype(np.float32))
        qp = (xflat @ M).reshape(B, S, D)
        in_maps.append({
            "xt8": xt8,
            "qt8": _to_planes8(qp),
            "xn": xn16,
        })

    nc = _get_nc()
    try:
        try:
            res = run_bass_kernel_spmd(nc, in_maps, list(range(H)), trace=trace)
        except Exception:
            res = run_bass_kernel_spmd(nc, in_maps, list(range(H)), trace=trace)
    except Exception:
        # device unusable after a retry: fall back to (slow) host math
        return _numpy_fallback(emb, Wq, bq, Wk, bk, Wv, bv, Wp, bp), None
    acc = np.zeros((B * S, D), dtype=np.float32)
    for h in range(H):
        # z: [128, gw, me, q] -> [gw, q, me*128+p] = [B*S rows, D]
        z = res.results[h]["z"].astype(np.float32)
        zt = z.transpose(1, 3, 2, 0).reshape(B * S, D)
        # r: [128, gw, q]: reduce key-partition lanes -> rows b*S + w*512 + q
        r = res.results[h]["r"].astype(np.float32)
        rv = r.sum(axis=0).reshape(B * S)
        acc += (zt / rv[:, None]) @ Gs[h]
    out = acc.reshape(B, S, D) + bp[None, None, :]
    return out.astype(np.float32), res


def kernel(**inputs):
    out, _ = _run(inputs, trace=False)
    return out
